# revision 1
# baseline (speedup 1.0000x reference)
"""Trainium2 Bass kernel: 2D parallel-beam forward projection (Radon transform).

Input:  x [2, 256, 256, 1] float32
Output: sinogram [2, 180, 363, 1] float32

Strategy (8 NeuronCores, SPMD):
  - Angles interleaved across cores (core c: angles c, c+8, ...); detectors
    interleaved across the 8 GPSIMD stream-groups (group g: d % 8 == g).
  - Each bilinear ray-sample is decomposed host-side into 1-2 "columns". A column
    points (via a gather index) at an 8-row-window x-pair table of the image and
    carries folded weights (y-hat x x-lerp) over 16 partition rows
    (2 batches x 8 window rows).
  - Device: GPSIMD ap_gather fetches x-pairs for all 16 partition rows per column,
    DVE computes W0*G0, W1*G1 and one fused segmented reduce per chunk,
    TensorE folds the 16 partition rows into per-(stream,batch) ray sums.
  - Rays are sorted by column count per stream; chunks pack a near-constant
    column count (variable rays x static per-chunk segment length); weight and
    index streams are stored chunk-major contiguous in HBM for line-rate DMA.
  - Host reassembles the full sinogram from the 8 per-core outputs.

All geometry tables/weights are functions of constants only (not of the image);
the only image-dependent device input is a layout-transformed copy of x.
"""
import os
import sys
from contextlib import ExitStack

import numpy as np

for p in ("/opt/trn_rl_repo", "/root/.axon_site/_ro/trn_rl_repo"):
    if os.path.isdir(p) and p not in sys.path:
        sys.path.insert(0, p)

import concourse.bass as bass  # noqa: E402,F401
import concourse.bacc as bacc  # noqa: E402
import concourse.mybir as mybir  # noqa: E402
import concourse.tile as tile  # noqa: E402
from concourse import bass_utils  # noqa: E402

F32 = mybir.dt.float32
I16 = mybir.dt.int16

# ---- geometry constants (mirror of the reference) ----
VOL = 256
N_ANGLES = 180
N_DET = 363
N_SAMPLES = 363
CEN = (VOL - 1) / 2.0
DCEN = (N_DET - 1) / 2.0
SCEN = (N_SAMPLES - 1) / 2.0

N_U = 33
N_XSLOT = 257
NUM_ELEMS = N_U * N_XSLOT  # 8481

N_CORES = 8
TARGET_COLS = 1792   # columns per chunk (approx)
IB = 8               # chunks per idx-batch DMA

_plan_cache = {}
_compile_cache = {}


def _f32(v):
    return np.float32(v)


def _ray_columns(theta32):
    """Column decomposition for one angle (float32 geometry to mirror jax)."""
    c = np.cos(theta32, dtype=np.float32)
    s = np.sin(theta32, dtype=np.float32)
    d = np.arange(N_DET, dtype=np.float32)[:, None]
    t = np.arange(N_SAMPLES, dtype=np.float32)[None, :]
    dc = (d - _f32(DCEN)).astype(np.float32)
    ts = (t - _f32(SCEN)).astype(np.float32)
    fx = (c * dc - s * ts + _f32(CEN)).astype(np.float32)
    fy = (s * dc + c * ts + _f32(CEN)).astype(np.float32)
    x0 = np.floor(fx).astype(np.int64)
    y0 = np.floor(fy).astype(np.int64)
    wx = (fx - x0).astype(np.float64)
    wy = (fy - y0).astype(np.float64)

    xslot = np.clip(x0, 0, 256)
    a0 = np.where((x0 >= 0) & (x0 < VOL), 1.0 - wx, 0.0)
    a1 = np.where((x0 + 1 >= 0) & (x0 + 1 < VOL), wx, 0.0)
    neg1 = x0 == -1
    a0 = np.where(neg1, wx, a0)
    a1 = np.where(neg1, 0.0, a1)
    x_dead = (x0 < -1) | (x0 > 255)

    lo_valid = (y0 >= 0) & (y0 < VOL) & ~x_dead
    hi_valid = (y0 + 1 >= 0) & (y0 + 1 < VOL) & ~x_dead
    u_lo = y0 >> 3
    m_lo = y0 & 7
    u_hi = (y0 + 1) >> 3
    m_hi = (y0 + 1) & 7
    same_window = m_lo <= 6

    colA = lo_valid | (hi_valid & same_window)
    colB = hi_valid & ~same_window

    D_idx = np.broadcast_to(np.arange(N_DET)[:, None], fx.shape)
    out = []
    for mask, uu, wlo, use_lo, use_hi, whi in (
        (colA, u_lo, m_lo, lo_valid, hi_valid & same_window, m_lo + 1),
        (colB, u_hi, m_hi, np.zeros_like(lo_valid), colB, m_hi),
    ):
        idx = np.nonzero(mask)
        if len(idx[0]) == 0:
            continue
        ylo = np.where(use_lo[idx], 1.0 - wy[idx], 0.0)
        yhi = np.where(use_hi[idx], wy[idx], 0.0)
        out.append((D_idx[idx], uu[idx], xslot[idx], wlo[idx], ylo,
                    np.where(use_hi[idx], whi[idx], 0), yhi, a0[idx], a1[idx]))
    cat = [np.concatenate([o[k] for o in out]) for k in range(9)]
    return cat  # ray(d), u, xslot, w_lo, y_lo, w_hi, y_hi, a0, a1


def _build_plan():
    """Geometry-only precompute shared across calls."""
    if "plan" in _plan_cache:
        return _plan_cache["plan"]
    cols = {}
    cnts = np.zeros((N_ANGLES, N_DET), dtype=np.int64)
    for a in range(N_ANGLES):
        theta = np.float32(a) * _f32(np.pi / N_ANGLES)
        cat = _ray_columns(theta)
        cols[a] = cat
        cnts[a] = np.bincount(cat[0], minlength=N_DET)

    core_ids = [list(range(c, N_ANGLES, N_CORES)) for c in range(N_CORES)]

    stream_rays = {}
    NR0 = 0
    for c in range(N_CORES):
        for g in range(8):
            ids = core_ids[c]
            ds = np.arange(g, N_DET, 8)
            A, D = np.meshgrid(ids, ds, indexing="ij")
            A, D = A.ravel(), D.ravel()
            lens = cnts[A, D]
            o = np.argsort(-lens, kind="stable")
            stream_rays[(c, g)] = (A[o], D[o], lens[o])
            NR0 = max(NR0, len(A))

    # global sorted length profile
    P = np.zeros(NR0, dtype=np.int64)
    for (c, g), (A, D, lens) in stream_rays.items():
        P[:len(lens)] = np.maximum(P[:len(lens)], lens)

    # chunk schedule: variable CH, L multiple of 16, ~TARGET_COLS per chunk
    chunks = []   # (p0, CHk, Lk, coloff, idxoff_int16)
    coloff = 0
    p = 0
    while p < NR0 and P[p] > 0:
        Lk = int((P[p] + 15) // 16 * 16)
        CHk = max(1, TARGET_COLS // Lk)
        CHk = min(CHk, NR0 - p)
        chunks.append((p, CHk, Lk, coloff))
        coloff += CHk * Lk
        p += CHk
    NR = p                    # positions covered by chunks (rest are empty rays)
    NRall = NR0
    Ntot = coloff

    # per-position column start
    colstart = np.zeros(NRall, dtype=np.int64)
    Lof = np.zeros(NRall, dtype=np.int64)
    for (p0, CHk, Lk, off) in chunks:
        for r in range(CHk):
            colstart[p0 + r] = off + r * Lk
            Lof[p0 + r] = Lk

    cores = []
    for c in range(N_CORES):
        ids = core_ids[c]
        posmap = np.full((8, N_ANGLES, N_DET // 8 + 1), -1, dtype=np.int64)
        for g in range(8):
            A, D, lens = stream_rays[(c, g)]
            posmap[g, A, D // 8] = np.arange(len(A))
        idxs = np.zeros((8, Ntot), dtype=np.int16)
        W08 = np.zeros((8, 8, Ntot), dtype=np.float32)
        W18 = np.zeros((8, 8, Ntot), dtype=np.float32)
        for a in ids:
            ray, uu, xs, wlo, ylo, whi, yhi, a0w, a1w = cols[a]
            g_of = ray % 8
            order = np.argsort(ray, kind="stable")
            r_o = ray[order]
            pos_in_ray = np.arange(len(r_o)) - np.searchsorted(r_o, r_o)
            g_o = g_of[order]
            p_o = posmap[g_o, a, r_o // 8]
            col = colstart[p_o] + pos_in_ray
            gi = (uu[order] * N_XSLOT + xs[order]).astype(np.int16)
            idxs[g_o, col] = gi
            ylo_o, yhi_o = ylo[order], yhi[order]
            a0_o, a1_o = a0w[order], a1w[order]
            wlo_o, whi_o = wlo[order], whi[order]
            W08[g_o, wlo_o, col] = (ylo_o * a0_o).astype(np.float32)
            W18[g_o, wlo_o, col] = (ylo_o * a1_o).astype(np.float32)
            vh = yhi_o != 0
            W08[g_o[vh], whi_o[vh], col[vh]] = (yhi_o[vh] * a0_o[vh]).astype(np.float32)
            W18[g_o[vh], whi_o[vh], col[vh]] = (yhi_o[vh] * a1_o[vh]).astype(np.float32)

        W0 = np.zeros((128, Ntot), dtype=np.float32)
        W1 = np.zeros((128, Ntot), dtype=np.float32)
        for g in range(8):
            for b in range(2):
                W0[16 * g + 8 * b:16 * g + 8 * b + 8] = W08[g]
                W1[16 * g + 8 * b:16 * g + 8 * b + 8] = W18[g]
        idxw = np.zeros((128, Ntot // 16), dtype=np.int16)
        for g in range(8):
            idxw[16 * g:16 * g + 16, :] = idxs[g].reshape(Ntot // 16, 16).T

        # repack chunk-major contiguous: w01flat, idxflat
        w01flat = np.empty(128 * 2 * Ntot, dtype=np.float32)
        idxflat = np.empty(128 * (Ntot // 16), dtype=np.int16)
        for (p0, CHk, Lk, off) in chunks:
            NCH = CHk * Lk
            blk = np.concatenate([W0[:, off:off + NCH], W1[:, off:off + NCH]], axis=1)
            w01flat[2 * 128 * off:2 * 128 * (off + NCH)] = blk.reshape(-1)
        # idx: batch-major blocks of IB chunks (must mirror the kernel's batching)
        ioff = 0
        k = 0
        while k < len(chunks):
            ke = min(k + IB, len(chunks))
            o0 = chunks[k][3]
            o1 = chunks[ke - 1][3] + chunks[ke - 1][1] * chunks[ke - 1][2]
            iblk = idxw[:, o0 // 16:o1 // 16]
            n16 = iblk.shape[1]
            idxflat[ioff:ioff + 128 * n16] = iblk.reshape(-1)
            ioff += 128 * n16
            k = ke
        raymap = []
        for g in range(8):
            A, D, lens = stream_rays[(c, g)]
            raymap.append((A, D))
        cores.append(dict(idxflat=idxflat, w01flat=w01flat, raymap=raymap))

    ones = np.zeros((128, 16), dtype=np.float32)
    for g in range(8):
        for b in range(2):
            ones[16 * g + 8 * b:16 * g + 8 * b + 8, 2 * g + b] = 1.0

    plan = dict(chunks=chunks, NR=NRall, Ntot=Ntot, cores=cores, ones=ones)
    _plan_cache["plan"] = plan
    return plan


def _build_tables(x):
    """x [2,256,256] -> tbl [128, NUM_ELEMS*2] f32 (pure layout transform)."""
    img = np.zeros((2, 8 * N_U + 8, N_XSLOT + 1), dtype=np.float32)
    img[:, :VOL, :VOL] = x
    tbl = np.zeros((16, NUM_ELEMS, 2), dtype=np.float32)
    for b in range(2):
        for w in range(8):
            rows = img[b, w:w + 8 * N_U:8, :]
            pair = np.stack([rows[:, :N_XSLOT], rows[:, 1:N_XSLOT + 1]], axis=-1)
            tbl[8 * b + w] = pair.reshape(NUM_ELEMS, 2)
    return np.tile(tbl, (8, 1, 1)).reshape(128, NUM_ELEMS * 2)


def _radon_kernel(tc, outs, ins, *, chunks, NR):
    ctx = ExitStack()
    with ctx:
        nc = tc.nc
        tbl_d, idx_d, w01_d, ones_d = ins
        out16_d = outs[0]

        const_pool = ctx.enter_context(tc.tile_pool(name="const", bufs=1))
        g_pool = ctx.enter_context(tc.tile_pool(name="g", bufs=2))
        w_pool = ctx.enter_context(tc.tile_pool(name="w", bufs=4))
        t_pool = ctx.enter_context(tc.tile_pool(name="t", bufs=2))
        i_pool = ctx.enter_context(tc.tile_pool(name="i", bufs=2))
        acc_pool = ctx.enter_context(tc.tile_pool(name="acc", bufs=1))
        psum_pool = ctx.enter_context(tc.tile_pool(name="ps", bufs=2, space="PSUM"))

        tbl = const_pool.tile([128, NUM_ELEMS * 2], F32)
        nc.sync.dma_start(tbl[:], tbl_d[:])
        ones = const_pool.tile([128, 16], F32)
        nc.sync.dma_start(ones[:], ones_d[:])

        acc = acc_pool.tile([128, NR], F32)
        nc.vector.memset(acc[:], 0.0)

        # idx batches: IB chunks per DMA
        nchunks = len(chunks)
        batches = []
        k = 0
        ioff = 0
        while k < nchunks:
            ke = min(k + IB, nchunks)
            n16 = sum(chunks[j][1] * chunks[j][2] // 16 for j in range(k, ke))
            batches.append((k, ke, ioff, n16))
            ioff += 128 * n16
            k = ke

        for (k0, k1, ioff, n16) in batches:
            ib = i_pool.tile([128, n16], I16, tag="idx")
            nc.sync.dma_start(
                ib[:], idx_d[ioff:ioff + 128 * n16].rearrange("(p n) -> p n", p=128))
            sl = 0
            for j in range(k0, k1):
                p0, CHk, Lk, off = chunks[j]
                NCH = CHk * Lk
                w01 = w_pool.tile([128, 2 * NCH], F32, tag="w01")
                nc.sync.dma_start(
                    w01[:],
                    w01_d[2 * 128 * off:2 * 128 * (off + NCH)]
                    .rearrange("(p n) -> p n", p=128))

                gt = g_pool.tile([128, NCH * 2], F32, tag="g")
                nc.gpsimd.ap_gather(
                    gt[:], tbl[:], ib[:, sl:sl + NCH // 16],
                    channels=128, num_elems=NUM_ELEMS, d=2, num_idxs=NCH,
                )
                g4 = gt[:].rearrange("p (r l two) -> p r l two", two=2, l=Lk)
                t01 = t_pool.tile([128, 2 * NCH], F32, tag="t01")
                t4 = t01[:].rearrange("p (r two l) -> p r two l", two=2, l=Lk)
                w4 = w01[:].rearrange("p (two r l) -> p two r l", two=2, l=Lk)
                nc.vector.tensor_mul(t4[:, :, 0, :], w4[:, 0, :, :], g4[:, :, :, 0])
                nc.vector.tensor_mul(t4[:, :, 1, :], w4[:, 1, :, :], g4[:, :, :, 1])
                nc.vector.tensor_reduce(
                    acc[:, p0:p0 + CHk],
                    t01[:].rearrange("p (r tl) -> p r tl", tl=2 * Lk),
                    axis=mybir.AxisListType.X,
                    op=mybir.AluOpType.add,
                )
                sl += NCH // 16

        outs_sb = acc_pool.tile([16, NR], F32)
        NMM = 512
        for m0 in range(0, NR, NMM):
            m1 = min(m0 + NMM, NR)
            ps = psum_pool.tile([16, m1 - m0], F32)
            nc.tensor.matmul(ps[:], ones[:], acc[:, m0:m1], start=True, stop=True)
            nc.scalar.copy(outs_sb[:, m0:m1], ps[:])
        nc.sync.dma_start(out16_d[:], outs_sb[:])


def _compile(plan):
    key = "nc"
    if key in _compile_cache:
        return _compile_cache[key]
    Ntot, NR = plan["Ntot"], plan["NR"]
    nc = bacc.Bacc("TRN2", target_bir_lowering=False, debug=False,
                   enable_asserts=False, num_devices=N_CORES)
    tbl_d = nc.dram_tensor("tbl", [128, NUM_ELEMS * 2], F32, kind="ExternalInput").ap()
    idx_d = nc.dram_tensor("idxf", [128 * (Ntot // 16)], I16, kind="ExternalInput").ap()
    w01_d = nc.dram_tensor("w01f", [128 * 2 * Ntot], F32, kind="ExternalInput").ap()
    ones_d = nc.dram_tensor("ones", [128, 16], F32, kind="ExternalInput").ap()
    out16_d = nc.dram_tensor("out16", [16, NR], F32, kind="ExternalOutput").ap()
    with tile.TileContext(nc) as tc:
        _radon_kernel(tc, [out16_d], [tbl_d, idx_d, w01_d, ones_d],
                      chunks=plan["chunks"], NR=plan["NR"])
    nc.compile()
    _compile_cache[key] = nc
    return nc


def kernel(x):
    """x [2,256,256,1] f32 -> sinogram [2,180,363,1] f32."""
    x = np.asarray(x, dtype=np.float32)
    plan = _build_plan()
    tbl = _build_tables(x[:, :, :, 0])
    nc = _compile(plan)
    in_maps = []
    for c in range(N_CORES):
        st = plan["cores"][c]
        in_maps.append(dict(tbl=tbl, idxf=st["idxflat"], w01f=st["w01flat"],
                            ones=plan["ones"]))
    res = bass_utils.run_bass_kernel_spmd(nc, in_maps, core_ids=list(range(N_CORES)))
    sino = np.zeros((2, N_ANGLES, N_DET), dtype=np.float32)
    for c in range(N_CORES):
        out16 = res.results[c]["out16"]
        for g in range(8):
            A, D = plan["cores"][c]["raymap"][g]
            n = len(A)
            for b in range(2):
                sino[b, A, D] = out16[2 * g + b, :n]
    return sino[..., None]


if __name__ == "__main__":
    import time
    x = np.load("/tmp/x.npy")
    t0 = time.time()
    out = kernel(x)
    print("kernel() wall time:", time.time() - t0)
    exp = np.load("/tmp/expected_np.npy")
    rel = np.linalg.norm((out - exp).ravel()) / np.linalg.norm(exp.ravel())
    print("rel l2 vs numpy ref:", rel)

